# revision 19
# baseline (speedup 1.0000x reference)
"""MoE top-2 routing kernel for Trainium2 (8 NeuronCores, batch-sharded).

Problem (hardcoded shapes):
    x [8192, 3072] f32, Wg [3072, 8], bg [8], W1 [8, 3072, 128], b1 [8, 128],
    W2 [8, 128, 10], b2 [8, 10]  ->  out [8192, 10] f32
    g = x@Wg + bg; top-2 softmax over selected logits;
    y = sum_k w_k * (relu(x@W1[e_k] + b1[e_k]) @ W2[e_k] + b2[e_k])

v3 design (per core, 1024 tokens = 2 tiles x 512, dense over experts):
  - Expert/gating/combine matmuls all bf16 (1 cyc/row on PE, same as
    f32r): W1/W2/Wg/b2 arrive as bf16 via gpsimd casting DMAs -- W1
    traffic halves to 6.3 MB so tile 0 is no longer DMA-starved.
  - x split once per chunk into an exact bf16 hi/lo pair off the fp32
    PE transpose: xh = bf16(xT) (ACT PSUM->SBUF copy), xl = bf16(xT-xh)
    (DVE mixed-dtype subtract).
  - Gating is 3-pass bf16 (xh@wgh + xl@wgh + xh@wgl): logit error
    ~2^-17, top-2 flips vs fp32 essentially zero.  Experts use xh only
    (~3e-3 rel err, well under the 2e-2 gate).
  - Experts in 8 single-expert waves per tile; wave 1 carries the
    transposes + gating; expert e's combine is injected into wave e+1.
  - Top-2 epilogue in [8, 512] orientation via gpsimd
    partition_all_reduce(max) + is_equal masks; per-token weights are
    broadcast across partitions with a PE rank-1 outer product (ones x
    wT-row staged to partition 0 by a tiny SBUF->SBUF DMA), applied to
    hr on DVE, and the 8 weighted W2 matmuls accumulate in one y PSUM
    bank (plus a b2^T @ wT matmul for the bias mix).
  - DMA: x in 4-chunk blocks (2 KB runs) on the sync queue; W1/consts
    casting DMAs on gpsimd (SWDGE); out + wg_f on the scalar queue.
"""
import sys

for _p in ("/opt/trn_rl_repo",):
    if _p not in sys.path:
        sys.path.insert(0, _p)

import numpy as np
from contextlib import ExitStack

import concourse.bass as bass
import concourse.bacc as bacc
import concourse.bass_isa as bass_isa
import concourse.tile as tile
import concourse.mybir as mybir
from concourse import bass_utils, masks

F32 = mybir.dt.float32
BF16 = mybir.dt.bfloat16
AF = mybir.ActivationFunctionType
OP = mybir.AluOpType

B, D, H, O, NE = 8192, 3072, 128, 10, 8
NCORES = 8
BC = B // NCORES          # tokens per core
TT = 512                  # token tile
NT = BC // TT             # token tiles per core
NCH = D // 128            # contraction chunks
NG = TT // 128            # 128-token groups per tile
XB = 4                    # x chunks per DMA block

_CACHE = {}


def _build_program():
    nc = bacc.Bacc("TRN2", target_bir_lowering=False, debug=False,
                   num_devices=NCORES)

    x = nc.dram_tensor("x", [BC, D], F32, kind="ExternalInput").ap()
    wg = nc.dram_tensor("Wg", [D, NE], F32, kind="ExternalInput").ap()
    bg = nc.dram_tensor("bg", [NE], F32, kind="ExternalInput").ap()
    w1 = nc.dram_tensor("W1", [NE, D, H], F32, kind="ExternalInput").ap()
    b1 = nc.dram_tensor("b1", [NE, H], F32, kind="ExternalInput").ap()
    w2 = nc.dram_tensor("W2", [NE, H, O], F32, kind="ExternalInput").ap()
    b2 = nc.dram_tensor("b2", [NE, O], F32, kind="ExternalInput").ap()
    out = nc.dram_tensor("out", [BC, O], F32, kind="ExternalOutput").ap()

    with tile.TileContext(nc) as tc:
        with ExitStack() as ctx:
            _kernel_body(ctx, tc, nc, x, wg, bg, w1, b1, w2, b2, out)
    nc.compile()
    return nc


def _kernel_body(ctx, tc, nc, x, wg, bg, w1, b1, w2, b2, out):
    singles = ctx.enter_context(tc.tile_pool(name="singles", bufs=1))
    xin_p = ctx.enter_context(tc.tile_pool(name="xin", bufs=3))
    xtr_p = ctx.enter_context(tc.tile_pool(name="xtr", bufs=NCH))
    xlo_p = ctx.enter_context(tc.tile_pool(name="xlo", bufs=3))
    gate_p = ctx.enter_context(tc.tile_pool(name="gate", bufs=8))
    wt_p = ctx.enter_context(tc.tile_pool(name="wt", bufs=2))
    hr_p = ctx.enter_context(tc.tile_pool(name="hr", bufs=2))
    hrw_p = ctx.enter_context(tc.tile_pool(name="hrw", bufs=2))
    wrow_p = ctx.enter_context(tc.tile_pool(name="wrow", bufs=2))
    yout_p = ctx.enter_context(tc.tile_pool(name="yout", bufs=2))

    ps_xtp = ctx.enter_context(tc.tile_pool(name="ps_xtp", bufs=2, space="PSUM"))
    ps_g = ctx.enter_context(tc.tile_pool(name="ps_g", bufs=1, space="PSUM"))
    ps_h = ctx.enter_context(tc.tile_pool(name="ps_h", bufs=3, space="PSUM"))
    ps_y = ctx.enter_context(tc.tile_pool(name="ps_y", bufs=1, space="PSUM"))
    ps_wb = ctx.enter_context(tc.tile_pool(name="ps_wb", bufs=1, space="PSUM"))

    # ---- constants ----
    ident = singles.tile([128, 128], F32)
    masks.make_identity(nc, ident[:])
    ones_f = singles.tile([1, 128], F32)
    nc.vector.memset(ones_f[:], 1.0)
    ones_b = singles.tile([1, 128], BF16)
    nc.vector.tensor_copy(ones_b[:], ones_f[:])

    bg_sb = singles.tile([NE, 1], F32)
    nc.scalar.dma_start(bg_sb[:], bg.rearrange("(e one) -> e one", one=1))
    b1t_sb = singles.tile([H, NE], F32)
    nc.scalar.dma_start(b1t_sb[:], b1.rearrange("e h -> h e"))

    # bf16 weights via gpsimd casting DMAs (SWDGE converts f32 -> bf16)
    wg_h = singles.tile([128, NCH, NE], BF16)
    nc.gpsimd.dma_start(wg_h[:], wg.rearrange("(c j) e -> j c e", j=128))
    wg_f = singles.tile([128, NCH, NE], F32)
    nc.scalar.dma_start(wg_f[:], wg.rearrange("(c j) e -> j c e", j=128))
    wg_l = singles.tile([128, NCH, NE], BF16)
    nc.vector.tensor_tensor(wg_l[:], wg_f[:], wg_h[:], op=OP.subtract)
    w2_b = singles.tile([H, NE, O], BF16)
    nc.gpsimd.dma_start(w2_b[:], w2.rearrange("e h o -> h e o"))
    b2_b = singles.tile([NE, O], BF16)
    nc.gpsimd.dma_start(b2_b[:], b2)
    w1_b = []
    for e in range(NE):
        w1e = singles.tile([128, NCH, H], BF16, tag=f"w1_{e}", name=f"w1_{e}")
        nc.gpsimd.dma_start(w1e[:], w1[e].rearrange("(c j) h -> j c h", j=128))
        w1_b.append(w1e)

    # ---- per token tile ----
    for t in range(NT):
        tok0 = t * TT

        xhs = [
            xtr_p.tile([128, TT], BF16, tag="xtr", name=f"xh{t}_{c}")
            for c in range(NCH)
        ]
        xls = {}
        xblks = {}
        g_ps = ps_g.tile([NE, TT], F32, tag="g")

        def transpose_chunk(c):
            # x arrives in XB-chunk blocks (2 KB per-partition runs ->
            # near-peak DMA efficiency); transposes slice the block
            b, ci = divmod(c, XB)
            if ci == 0:
                xin = xin_p.tile([128, NG, XB * 128], F32, tag="xin")
                nc.sync.dma_start(
                    xin[:],
                    x[
                        tok0 : tok0 + TT, b * XB * 128 : (b + 1) * XB * 128
                    ].rearrange("(gg p) d -> p gg d", p=128),
                )
                xblks[b] = xin
            xin = xblks[b]
            xtp = ps_xtp.tile([128, TT], F32, tag="xtp")
            for gg in range(NG):
                nc.tensor.matmul(
                    xtp[:, gg * 128 : (gg + 1) * 128],
                    xin[:, gg, ci * 128 : (ci + 1) * 128],
                    ident[:],
                    is_transpose=True,
                    start=True,
                    stop=True,
                    skip_group_check=True,
                )
            # exact bf16 split: xh = bf16(xT) on ACT; xl = bf16(xT - xh)
            nc.scalar.copy(xhs[c][:], xtp[:])
            xl = xlo_p.tile([128, TT], BF16, tag="xlo")
            nc.vector.tensor_tensor(xl[:], xtp[:], xhs[c][:], op=OP.subtract)
            xls[c] = xl

        def gating(c):
            nc.tensor.matmul(
                g_ps[:], wg_h[:, c, :], xhs[c][:],
                start=(c == 0), stop=False,
            )
            nc.tensor.matmul(
                g_ps[:], wg_h[:, c, :], xls.pop(c)[:],
                start=False, stop=False,
            )
            nc.tensor.matmul(
                g_ps[:], wg_l[:, c, :], xhs[c][:],
                start=False, stop=(c == NCH - 1),
            )

        def h_matmul(e, h_ps, c):
            nc.tensor.matmul(
                h_ps[:],
                w1_b[e][:, c],
                xhs[c][:],
                start=(c == 0),
                stop=(c == NCH - 1),
            )

        # ---- wave 1 (expert 0) carries transposes + gating ----
        h_cur = ps_h.tile([128, TT], F32, tag="h", name=f"h{t}_0")
        for c in range(NCH):
            transpose_chunk(c)
            if c >= 2:
                gating(c - 2)
            if c >= 1:
                h_matmul(0, h_cur, c - 1)
        gating(NCH - 2)
        gating(NCH - 1)
        h_matmul(0, h_cur, NCH - 1)

        # ---- gating epilogue (DVE/ACT/gpsimd; no PE) -> wT [8, TT] ----
        g_sb = gate_p.tile([NE, TT], F32, tag="ge")
        nc.vector.tensor_scalar(g_sb[:], g_ps[:], bg_sb[:, 0:1], None, OP.add)
        m1 = gate_p.tile([NE, TT], F32, tag="ge")
        nc.gpsimd.partition_all_reduce(
            m1[:], g_sb[:], channels=NE, reduce_op=bass_isa.ReduceOp.max
        )
        eq1 = gate_p.tile([NE, TT], F32, tag="ge")
        nc.vector.tensor_tensor(eq1[:], g_sb[:], m1[:], op=OP.is_equal)
        negb = gate_p.tile([NE, TT], F32, tag="ge")
        nc.vector.tensor_scalar(negb[:], eq1[:], -1e30, None, OP.mult)
        g2 = gate_p.tile([NE, TT], F32, tag="ge")
        nc.vector.tensor_tensor(g2[:], g_sb[:], negb[:], op=OP.add)
        m2 = gate_p.tile([NE, TT], F32, tag="ge")
        nc.gpsimd.partition_all_reduce(
            m2[:], g2[:], channels=NE, reduce_op=bass_isa.ReduceOp.max
        )
        eq2 = gate_p.tile([NE, TT], F32, tag="ge")
        nc.vector.tensor_tensor(eq2[:], g_sb[:], m2[:], op=OP.is_equal)
        d21 = gate_p.tile([NE, TT], F32, tag="ge")
        nc.vector.tensor_tensor(d21[:], m2[:], m1[:], op=OP.subtract)
        ex = gate_p.tile([NE, TT], F32, tag="ge")
        nc.scalar.activation(ex[:], d21[:], AF.Exp)
        den = gate_p.tile([NE, TT], F32, tag="ge")
        nc.vector.tensor_scalar(den[:], ex[:], 1.0, None, OP.add)
        rcp = gate_p.tile([NE, TT], F32, tag="ge")
        nc.vector.reciprocal(rcp[:], den[:])
        t1 = gate_p.tile([NE, TT], F32, tag="ge")
        nc.vector.tensor_tensor(t1[:], eq1[:], rcp[:], op=OP.mult)
        w2v = gate_p.tile([NE, TT], F32, tag="ge")
        nc.vector.tensor_tensor(w2v[:], ex[:], rcp[:], op=OP.mult)
        t2 = gate_p.tile([NE, TT], F32, tag="ge")
        nc.vector.tensor_tensor(t2[:], eq2[:], w2v[:], op=OP.mult)
        wT = wt_p.tile([NE, TT], BF16, tag="wt")
        nc.vector.tensor_tensor(wT[:], t1[:], t2[:], op=OP.add)

        y_ps = ps_y.tile([O, TT], F32, tag="y")
        n_acc = [0]

        def relu_expert(e, h_ps):
            hr = hr_p.tile([128, TT], BF16, tag="hr", name=f"hr{t}_{e}")
            nc.scalar.activation(
                hr[:], h_ps[:], AF.Relu, bias=b1t_sb[:, e : e + 1]
            )
            return hr

        def combine_expert(e, hr):
            # wT[e] row -> partition 0 (DMA remap), then broadcast across
            # partitions via PE rank-1 outer product
            wrow = wrow_p.tile([1, TT], BF16, tag="wrow", name=f"wrow{t}_{e}")
            nc.scalar.dma_start(wrow[:], wT[e : e + 1, :])
            wbc = ps_wb.tile([128, TT], F32, tag="wbc", name=f"wbc{t}_{e}")
            nc.tensor.matmul(
                wbc[:], ones_b[:], wrow[:], start=True, stop=True
            )
            hrw = hrw_p.tile([128, TT], BF16, tag="hrw", name=f"hrw{t}_{e}")
            nc.vector.tensor_tensor(hrw[:], wbc[:], hr[:], op=OP.mult)
            nc.tensor.matmul(
                y_ps[:],
                w2_b[:, e, :],
                hrw[:],
                start=(n_acc[0] == 0),
                stop=False,
            )
            n_acc[0] += 1

        # ---- waves 2..8 (experts 1..7); combine(e-1) injected into wave e
        pend = (0, relu_expert(0, h_cur))
        for e in range(1, NE):
            h_nxt = ps_h.tile([128, TT], F32, tag="h", name=f"h{t}_{e}")
            for c in range(NCH):
                h_matmul(e, h_nxt, c)
                if c == 10 and pend is not None:
                    combine_expert(pend[0], pend[1])
                    pend = None
            pend = (e, relu_expert(e, h_nxt))
        combine_expert(pend[0], pend[1])

        # b2 contribution: sum_e wT[e,t] * b2[e,:]
        nc.tensor.matmul(
            y_ps[:], b2_b[:], wT[:], start=False, stop=True,
        )

        # ---- output: [O, TT] -> token-major [TT, O] ----
        y_sb = yout_p.tile([O, TT], F32, tag="ysb")
        nc.vector.tensor_copy(y_sb[:], y_ps[:])
        # ride the xtp slot ring (same [128, TT] f32 shape) — no extra bank
        yt_ps = ps_xtp.tile([128, TT], F32, tag="xtp", name=f"ytps{t}")
        for gg in range(NG):
            nc.tensor.matmul(
                yt_ps[:, gg * O : (gg + 1) * O],
                y_sb[:, gg * 128 : (gg + 1) * 128],
                ident[0:O, 0:O],
                is_transpose=True,
                start=True,
                stop=True,
                skip_group_check=True,
            )
        yt_sb = yout_p.tile([128, NG * O], F32, tag="ytsb")
        nc.vector.tensor_copy(yt_sb[:], yt_ps[:, 0 : NG * O])
        nc.scalar.dma_start(
            out[tok0 : tok0 + TT].rearrange("(gg p) o -> p gg o", p=128),
            yt_sb[:].rearrange("p (gg o) -> p gg o", gg=NG),
        )


def _get_nc():
    if "nc" not in _CACHE:
        _CACHE["nc"] = _build_program()
    return _CACHE["nc"]


def kernel(x, Wg, bg, W1, b1, W2, b2, _trace=False, _tmpdir=None):
    nc = _get_nc()
    x = np.ascontiguousarray(np.asarray(x, dtype=np.float32))
    shared = {
        "Wg": np.ascontiguousarray(np.asarray(Wg, dtype=np.float32)),
        "bg": np.ascontiguousarray(np.asarray(bg, dtype=np.float32)),
        "W1": np.ascontiguousarray(np.asarray(W1, dtype=np.float32)),
        "b1": np.ascontiguousarray(np.asarray(b1, dtype=np.float32)),
        "W2": np.ascontiguousarray(np.asarray(W2, dtype=np.float32)),
        "b2": np.ascontiguousarray(np.asarray(b2, dtype=np.float32)),
    }
    in_maps = [
        {"x": x[c * BC : (c + 1) * BC], **shared} for c in range(NCORES)
    ]
    res = bass_utils.run_bass_kernel_spmd(
        nc,
        in_maps,
        core_ids=list(range(NCORES)),
        trace=_trace,
        tmpdir=_tmpdir,
    )
    outp = np.concatenate([res.results[c]["out"] for c in range(NCORES)], axis=0)
    if _trace:
        kernel._last_results = res
    return outp
